# revision 57
# baseline (speedup 1.0000x reference)
"""
DPCA3D sparse-attention kernel for 8 TRN2 NeuronCores (Bass/Tile).

Sharding: batch*heads (16 units) across 8 cores -> 2 heads of one batch per
core. The small 1x1-conv weights are folded on host; per-core tensors ship
pre-packed.

Device (per core, one NEFF, no collectives) computes ONLY the O(N*NKV) body:
  sim   = khat^T qhat per head over the 512 selected kv positions, as fp8
          DoubleRow matmuls (contraction 128 = 64 partitions x 2 k-tiles,
          one k-tile per head with the other head's block zeroed, so both
          heads share one packed qhat rhs) -> 2x PE throughput;
  exp   split across ACT and DVE (GPSIMD cannot access PSUM on real TRN2):
          exact Exp on ACT for ~63% of [128,1024] psum tiles, Schraudolph
          bit-trick exp (y = sim*128*log2e + c2 -> uint16 viewed as bf16,
          max rel err ~3%) via one tensor_scalar on DVE for the rest --
          host-prototyped end-to-end error 6-8e-3 vs the 2e-2 budget;
  av    "flipped" matmuls: out [128 vox, 65] with lhsT = ex [kv,vox] tiles
          and rhs = vf [kv, 64 v-channels | ones-col] so the free dim is 65
          instead of 512 (PE cost is free-size-based) and the softmax
          denominator rides along as column 64;
  copy  avT psum -> bf16 stage: both heads of a chunk share one [128,1024]
          pav tile (head 1 bank-aligned at col 512; accumulation groups must
          not straddle 2KB PSUM banks) so ONE strided copy per chunk (DVE)
          drains both; DMA out every 2 chunks, per-chunk at the drain.
          Input DMAs spread across SP/gpsimd/ACT DGE queues so generation
          latencies overlap (fill) and slab arrivals outpace consumers.

Host (f32 numpy): everything O(N*C): LN folds, convs, l2 norms, top-k
selection (f32-exact like the baseline), fp8/bf16 packing, and the finish:
attn = u/den, z = W_out @ attn, cross-core head-sum, channel-LN, residual.
"""

import numpy as np
import ml_dtypes

import concourse.bass as bass
import concourse.bacc as bacc
import concourse.tile as tile
import concourse.mybir as mybir
from concourse.bass_utils import run_bass_kernel_spmd
from concourse._compat import with_exitstack

BF16 = mybir.dt.bfloat16
F32 = mybir.dt.float32
F8 = mybir.dt.float8e4
U16 = mybir.dt.uint16
bf16 = ml_dtypes.bfloat16
f8e4 = ml_dtypes.float8_e4m3

HEADS, DH, C = 8, 64, 128
D, H, W = 16, 32, 32
N = D * H * W            # 16384 voxels per batch
B = 2
NCORES = 8
KD = KH = KW = 8
NKV = KD * KH * KW       # 512 selected kv positions per head
VCH = 512                # vox chunk
NVC = N // VCH           # 32 chunks
KVC = 128                # kv chunk

LOG2E = float(np.log2(np.e))
SCH_C1 = 128.0 * LOG2E
SCH_C2 = float(127 * 128) - 128.0 * 0.043036

DR = mybir.MatmulPerfMode.DoubleRow
EXP = mybir.ActivationFunctionType.Exp


def _mk_engine_seq(quotas, total):
    """Bresenham-spread engine assignment sequence honoring quotas."""
    seq = []
    acc = {e: 0.0 for e in quotas}
    for _ in range(total):
        for e in quotas:
            acc[e] += quotas[e] / total
        pick = max(acc, key=lambda e: acc[e])
        acc[pick] -= 1.0
        seq.append(pick)
    return seq


# 256 exp tiles ([128, 512] each). HW constraint: GPSIMD/Pool cannot touch
# PSUM, so only ACT (612ns) and DVE (658ns) can consume sim psum tiles; the
# 64 av stage copies (441ns, psum reads) also go to DVE/ACT.
EXP_QUOTAS = {'act': 80, 'dve': 48}
LAG = 2            # chunk-heads of av delay behind sim/exp
PSIM_BUFS = 3
PAV_BUFS = 2
SBEX_BUFS = 8
SIM_PRIO = 0       # >0: emit sim matmuls with high_priority(offset)
GRAIN = 1024       # exp/psum tile grain: 512 or 1024
WARMUP = 0
AVPAIR = 1         # pair both heads of a chunk in one [128,1024] pav tile
                   # (head 1 at col 512: bank-aligned; a [128,520] packing is
                   # illegal -- av groups must not straddle 2KB PSUM banks)
TAILW = 0          # drain-bias window of the engine assignment (0 = off)
SEQ_ROT = 0        # rotate the engine-assignment sequence (schedule jitter)
DRAINW = 1         # chunk-heads at the end finished in split [128,512] halves
STARTW = 1         # chunk-heads at the start in split halves
AV_FIRST = 0       # emit av(i-LAG) before sim_exp(i)
QMODE = 0          # slab queue pattern; UOUT_Q: 0=sync 1=gpsimd
UOUT_Q = 0
SLAB0 = 1          # chunks covered by the first qh DMA
SWAPS = ()         # (idx, engine) overrides of the exp assignment sequence
LASTSPLIT = 0      # last chunk per-head drain: tested worse than paired copy
RINGPOOL = 1       # single shared 4-slot psum ring for sim+av tiles
AVPOS = 0          # av emission phase: 0=after each sim_exp, 1=before even, 2=batched after odd

_EXP_SEQ = None


def _exp_engine(idx):
    global _EXP_SEQ
    if _EXP_SEQ is None:
        total = sum(EXP_QUOTAS.values())
        _EXP_SEQ = _mk_engine_seq(EXP_QUOTAS, total)
        for ix, e in SWAPS:
            _EXP_SEQ[ix] = e
        if SEQ_ROT:
            _EXP_SEQ = _EXP_SEQ[SEQ_ROT:] + _EXP_SEQ[:SEQ_ROT]
        if TAILW:
            # drain bias: within the last TAILW tiles, run DVE's share first
            # and ACT's last -- ACT drains its queue earlier, so the final
            # exps (which gate the last avs) land on the idle engine
            tail = _EXP_SEQ[-TAILW:]
            _EXP_SEQ[-TAILW:] = (
                [e for e in tail if e != 'act'] + [e for e in tail if e == 'act'])
    return _EXP_SEQ[idx % len(_EXP_SEQ)]


# ----------------------------------------------------------------------------
# device program
# ----------------------------------------------------------------------------

@with_exitstack
def _device_kernel(ctx, tc, io):
    nc = tc.nc
    qh_d = io['qh']        # [64, NVC*1024] f8: qhat packed (j, head r, x)
    kf_d = io['kf']        # [64, 2048] f8: per head [kc][r][128kv], zero off-head
    vf_d = io['vf']        # [128, 520] bf16: per head 4 kc-blocks [128kv, 65]
    uout = io['uout']      # [128, NVC*520] bf16 out: u|den per (j, h, vb)

    cpool = ctx.enter_context(tc.tile_pool(name="consts", bufs=1))
    # issue the critical first loads on three different DGE queues so their
    # generation latencies overlap; head A's kf half lands first, vf (only
    # needed by the first av, much later) goes last on the ACT queue
    kf = cpool.tile([64, 2048], F8)
    nc.gpsimd.dma_start(kf[:, 0:1024], kf_d[:, 0:1024])
    qh = cpool.tile([64, NVC * 1024], F8)
    nc.sync.dma_start(qh[:, 0:SLAB0 * 1024], qh_d[:, 0:SLAB0 * 1024])
    nc.gpsimd.dma_start(kf[:, 1024:2048], kf_d[:, 1024:2048])
    vf = cpool.tile([128, 520], BF16)
    nc.scalar.dma_start(vf[:], vf_d[:])
    # remaining qh slabs (first ones small) so the pipeline starts early;
    # alternate SP/gpsimd DGE queues so slab arrivals outpace the consumers
    edges = [SLAB0, 2, 4, 8, 12, 16, 24, 32]
    edges = [e for e in edges if e > SLAB0 or e == SLAB0]
    edges = sorted(set([SLAB0] + [e for e in [2, 4, 8, 12, 16, 24, 32] if e > SLAB0]))
    for s in range(len(edges) - 1):
        lo, hi = edges[s] * 1024, edges[s + 1] * 1024
        if QMODE == 4:
            eng = nc.gpsimd
        elif QMODE == 5:
            eng = nc.gpsimd if s % 2 == 0 else nc.sync
        else:
            eng = nc.sync if s % 2 == 0 else nc.gpsimd
        eng.dma_start(qh[:, lo:hi], qh_d[:, lo:hi])

    # PE pstate warmup: the tensor engine ramps 0.65 -> 2.4 GHz over ~3us of
    # continuous execution. Dummy matmuls on a zeroed scratch tile fill the
    # initial DMA-wait window so the first real sims run at full clock.
    warm = cpool.tile([64, 512], F8)
    nc.gpsimd.memset(warm[:], 0)

    # Software pipeline: av(i) is emitted LAG chunk-heads behind sim/exp(i)
    # so PE's FIFO queue never head-blocks on an exp still in flight.
    if RINGPOOL:
        ring = ctx.enter_context(
            tc.tile_pool(name="ring", bufs=4, space="PSUM"))
        psim = pav = ring
    else:
        psim = ctx.enter_context(
            tc.tile_pool(name="psim", bufs=PSIM_BUFS, space="PSUM"))
        pav = ctx.enter_context(
            tc.tile_pool(name="pav", bufs=(1 if AVPAIR else PAV_BUFS),
                         space="PSUM"))
    with tc.tile_pool(name="sbex", bufs=SBEX_BUFS) as sbex, \
         tc.tile_pool(name="sbst", bufs=3) as sbst:
        exs = {}
        stage = [None]

        avt = [None]

        def emit_exp(eng, exsl, smsl):
            if eng == 'act':
                nc.scalar.activation(exsl, smsl, EXP)
            elif eng == 'dve':
                nc.vector.tensor_scalar(
                    exsl.bitcast(U16), smsl, SCH_C1, SCH_C2,
                    op0=mybir.AluOpType.mult, op1=mybir.AluOpType.add)
            else:
                nc.gpsimd.tensor_scalar(
                    exsl.bitcast(U16), smsl, SCH_C1, SCH_C2,
                    op0=mybir.AluOpType.mult, op1=mybir.AluOpType.add)

        def emit_sim_exp(i):
            j, h = divmod(i, 2)
            rhs = qh[:, j * 1024:(j + 1) * 1024].rearrange(
                "p (two x) -> p two x", two=2)
            kfh = kf[:, h * 1024:(h + 1) * 1024]
            ex = sbex.tile([128, 2048], BF16, tag="ex")
            exs[i] = ex
            if GRAIN == 512:
                # one [128, 512] psum tile (= 1 bank) per kv-chunk with its
                # own exp instr: deepest sim ring (6 slots), each slot frees
                # on a single exp
                for kc in range(4):
                    sm = psim.tile([128, 512], F32, tag="sim")
                    nc.tensor.matmul(
                        sm[:],
                        lhsT=kfh[:, kc * 256:(kc + 1) * 256].rearrange(
                            "p (two m) -> p two m", two=2),
                        rhs=rhs, perf_mode=DR)
                    emit_exp(_exp_engine(4 * i + kc),
                             ex[:, kc * 512:(kc + 1) * 512], sm[:])
            else:
                # [128, 1024] psum tiles (2 banks, kc pairs): one exp instr
                # per tile amortizes the psum/sbuf access init
                drain = i >= NVC * 2 - DRAINW or i < STARTW
                for t in range(2):
                    sm = psim.tile([128, 1024], F32, tag="sim")
                    for kk in range(2):
                        kc = 2 * t + kk
                        nc.tensor.matmul(
                            sm[:, kk * 512:(kk + 1) * 512],
                            lhsT=kfh[:, kc * 256:(kc + 1) * 256].rearrange(
                                "p (two m) -> p two m", two=2),
                            rhs=rhs, perf_mode=DR)
                    if drain:
                        # last chunk-heads: finish in [128, 512] halves spread
                        # over BOTH engines so the final av gate clears early
                        for kk in range(2):
                            emit_exp('act' if (2 * t + kk) % 2 == 0 else 'dve',
                                     ex[:, (2 * t + kk) * 512:
                                        (2 * t + kk + 1) * 512],
                                     sm[:, kk * 512:(kk + 1) * 512])
                    else:
                        emit_exp(_exp_engine(2 * i + t),
                                 ex[:, t * 1024:(t + 1) * 1024], sm[:])

        def emit_av(i):
            j, h = divmod(i, 2)
            if j % 2 == 0 and h == 0:
                stage[0] = sbst.tile([128, 1040], BF16, tag="stage", name="stage")
            ex = exs.pop(i)
            vfh = vf[:, h * 260:(h + 1) * 260]
            if AVPAIR:
                # both heads of chunk j share one [128, 1024] pav tile with
                # head h at column h*512: each 65-col accumulation group stays
                # inside one 2KB PSUM bank (groups must not straddle banks),
                # and ONE strided copy per chunk replaces two copies
                if h == 0:
                    avt[0] = pav.tile([128, 1024], F32,
                                      tag=("sim" if RINGPOOL else "av"),
                                      name="avt")
                av = avt[0][:, h * 512:h * 512 + 260]
            else:
                av = pav.tile([128, 260], F32, tag="av")
            for vb in range(4):
                for kc in range(4):
                    nc.tensor.matmul(
                        av[:, vb * 65:(vb + 1) * 65],
                        lhsT=ex[:, kc * 512 + vb * 128:kc * 512 + (vb + 1) * 128],
                        rhs=vfh[:, kc * 65:(kc + 1) * 65],
                        start=(kc == 0), stop=(kc == 3))
            off = (j % 2) * 520 + h * 260
            if AVPAIR:
                if LASTSPLIT and j == NVC - 1:
                    # final chunk: drain each head as soon as its avs land
                    off2 = (j % 2) * 520 + h * 260
                    nc.scalar.copy(stage[0][:, off2:off2 + 260],
                                   avt[0][:, h * 512:h * 512 + 260])
                    nc.sync.dma_start(
                        uout[:, j * 520 + h * 260:j * 520 + (h + 1) * 260],
                        stage[0][:, off2:off2 + 260])
                    return
                if h == 1:
                    src3 = avt[0][:, 0:1024].rearrange(
                        "p (two x) -> p two x", two=2)[:, :, 0:260]
                    dst3 = stage[0][:, (j % 2) * 520:(j % 2) * 520 + 520].rearrange(
                        "p (two x) -> p two x", two=2)
                    oeng = nc.gpsimd if UOUT_Q else nc.sync
                    if j >= NVC - 2:
                        # drain: copy on ACT (its queue empties first), DMA
                        # per chunk as soon as the copy lands
                        nc.scalar.copy(dst3, src3)
                        oeng.dma_start(
                            uout[:, j * 520:(j + 1) * 520],
                            stage[0][:, (j % 2) * 520:(j % 2) * 520 + 520])
                    else:
                        nc.vector.tensor_copy(dst3, src3)
                        if j % 2 == 1:
                            oeng.dma_start(
                                uout[:, (j - 1) * 520:(j + 1) * 520], stage[0][:])
                return
            if i >= NVC * 2 - 4:
                # drain: copy on ACT (its exp queue empties first) and DMA
                # each chunk-head slice as soon as its copy lands
                nc.scalar.copy(stage[0][:, off:off + 260], av[:])
                nc.sync.dma_start(
                    uout[:, j * 520 + h * 260:j * 520 + (h + 1) * 260],
                    stage[0][:, off:off + 260])
            else:
                nc.vector.tensor_copy(stage[0][:, off:off + 260], av[:])
                if j % 2 == 1 and h == 1:
                    nc.sync.dma_start(
                        uout[:, (j - 1) * 520:(j + 1) * 520], stage[0][:])

        NW = NVC * 2
        for i in range(NW):
            if (AVPOS == 1 and i % 2 == 0 and i >= LAG) or \
               (AV_FIRST and i >= LAG):
                emit_av(i - LAG)
            emit_sim_exp(i)
            if AVPOS == 2:
                if i % 2 == 1:
                    for k in (i - 1, i):
                        if k >= LAG:
                            emit_av(k - LAG)
            elif AVPOS == 1:
                if i % 2 == 1 and i >= LAG:
                    emit_av(i - LAG)
            elif not AV_FIRST and i >= LAG:
                emit_av(i - LAG)
        for i in range(NW - LAG, NW):
            emit_av(i)


def _build_program():
    nc = bacc.Bacc("TRN2", target_bir_lowering=False, debug=False,
                   num_devices=NCORES)
    io = {}

    def inp(name, shape, dt):
        io[name] = nc.dram_tensor(name, shape, dt, kind="ExternalInput").ap()

    inp('qh', [64, NVC * 1024], F8)
    inp('kf', [64, 2048], F8)
    inp('vf', [128, 520], BF16)
    io['uout'] = nc.dram_tensor('uout', [128, NVC * 520], BF16,
                                kind="ExternalOutput").ap()

    with tile.TileContext(nc) as tc:
        _device_kernel(tc, io)
    nc.compile()
    return nc


_NC = None


def _get_program():
    global _NC
    if _NC is None:
        _NC = _build_program()
    return _NC


# ----------------------------------------------------------------------------
# host side
# ----------------------------------------------------------------------------

def _host_prepare(inputs):
    f32 = np.float32
    qs = np.asarray(inputs['query_source'], f32).reshape(B, C, N)
    ctxf = np.asarray(inputs['context'], f32).reshape(B, C, N)
    w_q = np.asarray(inputs['w_q'], f32)
    w_kv = np.asarray(inputs['w_kv'], f32)
    cg = np.asarray(inputs['ctx_gamma'], f32).reshape(C)
    cb = np.asarray(inputs['ctx_beta'], f32).reshape(C)
    qg = np.asarray(inputs['qs_gamma'], f32).reshape(C)
    qb = np.asarray(inputs['qs_beta'], f32).reshape(C)

    w_k, w_v = w_kv[:HEADS * DH], w_kv[HEADS * DH:]

    # f32 reference-equivalent pipeline (LN -> conv -> l2norm -> topk)
    def chan_ln(x, g, b):
        m = x.mean(1, keepdims=True)
        v = x.var(1, keepdims=True)
        return g[None, :, None] * (x - m) / (np.sqrt(v) + f32(1e-6)) + b[None, :, None]

    ctx_ln = chan_ln(ctxf, cg, cb)
    qs_ln = chan_ln(qs, qg, qb)
    k = np.einsum('bcn,oc->bon', ctx_ln, w_k).reshape(B * HEADS, DH, N)
    q = np.einsum('bcn,oc->bon', qs_ln, w_q).reshape(B * HEADS, DH, N)
    v = np.einsum('bcn,oc->bon', ctx_ln, w_v).reshape(B * HEADS, DH, N)

    def l2n(x):
        nn = np.sqrt((x * x).sum(1, keepdims=True))
        return x / np.maximum(nn, f32(1e-12))

    qh, kh = l2n(q), l2n(k)
    qp = qh.sum(2)                               # [16, 64]
    kab = np.abs(kh).reshape(B * HEADS, DH, D, H, W)
    sd = np.einsum('bc,bcd->bd', qp, kab.sum((3, 4)))
    sh = np.einsum('bc,bch->bh', qp, kab.sum((2, 4)))
    sw = np.einsum('bc,bcw->bw', qp, kab.sum((2, 3)))

    def topk(s, kk):
        return np.argsort(-s, axis=1, kind='stable')[:, :kk]

    id_, ih_, iw_ = topk(sd, KD), topk(sh, KH), topk(sw, KW)
    flat = (id_[:, :, None, None] * (H * W) + ih_[:, None, :, None] * W
            + iw_[:, None, None, :]).reshape(B * HEADS, NKV)

    in_maps = []
    for core in range(NCORES):
        b = core // 4
        hA = (core % 4) * 2
        bhs = (b * HEADS + hA, b * HEADS + hA + 1)

        # qhat packed: [64, NVC*1024], col j*1024 + r*512 + x = qh[bh_r, :, j*512+x]
        qpk = np.empty((64, NVC, 2, VCH), f32)
        for r, bh in enumerate(bhs):
            qpk[:, :, r, :] = qh[bh].reshape(DH, NVC, VCH)
        qpk = qpk.reshape(64, NVC * 1024).astype(f8e4)

        # kf packed [64, 2048]: col h*1024 + kc*256 + r*128 + m; head h's khat
        # sits in k-tile slot r==h, the other slot is zero (shared-rhs trick)
        kfp = np.zeros((64, 2, 4, 2, KVC), f32)
        for r, bh in enumerate(bhs):
            kfp[:, r, :, r, :] = kh[bh][:, flat[bh]].reshape(DH, 4, KVC)
        kfp = kfp.reshape(64, 2048).astype(f8e4)

        # vf: per head 4 blocks [128 kv, 65]: cols h*260 + kc*65 + c
        vfp = np.zeros((128, 520), f32)
        for r, bh in enumerate(bhs):
            vsel = v[bh][:, flat[bh]]            # [64, 512]
            for kc in range(4):
                blk = vsel[:, kc * KVC:(kc + 1) * KVC].T   # [128 kv, 64]
                vfp[:, r * 260 + kc * 65: r * 260 + kc * 65 + 64] = blk
                vfp[:, r * 260 + kc * 65 + 64] = 1.0

        in_maps.append({
            'qh': qpk,
            'kf': kfp,
            'vf': vfp.astype(bf16),
        })
    return in_maps, qs


def _host_finish(results, inputs, qs):
    f32 = np.float32
    w_out = np.asarray(inputs['w_out'], f32)
    og = np.asarray(inputs['out_gamma'], f32).reshape(1, C, 1)
    ob = np.asarray(inputs['out_beta'], f32).reshape(1, C, 1)
    gamma = np.asarray(inputs['gamma'], f32).reshape(-1)[0]
    z = np.zeros((B, C, N), f32)
    for core in range(NCORES):
        b = core // 4
        hA = (core % 4) * 2
        u = results[core]['uout'].astype(f32)        # [128, NVC*520]
        u = u.reshape(128, NVC, 2, 4, 65)            # p, j, h, vb, c
        for h in range(2):
            uh = u[:, :, h, :, :]                    # [128, NVC, 4, 65]
            # vox = j*512 + vb*128 + p
            uh = uh.transpose(1, 2, 0, 3).reshape(N, 65)
            attn = uh[:, :64] / uh[:, 64:65]         # [N, 64]
            z[b] += w_out[:, (hA + h) * DH:(hA + h + 1) * DH] @ attn.T
    m = z.mean(1, keepdims=True)
    vv = z.var(1, keepdims=True)
    out = og * (z - m) / (np.sqrt(vv) + f32(1e-6)) + ob
    out = gamma * out + qs
    return out.reshape(B, C, D, H, W).astype(f32)


def kernel(**inputs):
    in_maps, qs = _host_prepare(inputs)
    nc = _get_program()
    res = run_bass_kernel_spmd(nc, in_maps, list(range(NCORES)))
    return _host_finish(res.results, inputs, qs)


if __name__ == '__main__':
    import reference
    ins = {k: np.asarray(v) for k, v in reference.setup_inputs().items()}
    out = kernel(**ins)
    print("kernel output:", out.shape, out.dtype)


# revision 58
# speedup vs baseline: 1.0111x; 1.0111x over previous
"""
DPCA3D sparse-attention kernel for 8 TRN2 NeuronCores (Bass/Tile).

Sharding: batch*heads (16 units) across 8 cores -> 2 heads of one batch per
core. The small 1x1-conv weights are folded on host; per-core tensors ship
pre-packed.

Device (per core, one NEFF, no collectives) computes ONLY the O(N*NKV) body:
  sim   = khat^T qhat per head over the 512 selected kv positions, as fp8
          DoubleRow matmuls (contraction 128 = 64 partitions x 2 k-tiles,
          one k-tile per head with the other head's block zeroed, so both
          heads share one packed qhat rhs) -> 2x PE throughput;
  exp   split across ACT and DVE (GPSIMD cannot access PSUM on real TRN2):
          exact Exp on ACT for ~63% of [128,1024] psum tiles, Schraudolph
          bit-trick exp (y = sim*128*log2e + c2 -> uint16 viewed as bf16,
          max rel err ~3%) via one tensor_scalar on DVE for the rest --
          host-prototyped end-to-end error 6-8e-3 vs the 2e-2 budget;
  av    "flipped" matmuls: out [128 vox, 65] with lhsT = ex [kv,vox] tiles
          and rhs = vf [kv, 64 v-channels | ones-col] so the free dim is 65
          instead of 512 (PE cost is free-size-based) and the softmax
          denominator rides along as column 64;
  copy  avT psum -> bf16 stage: both heads of a chunk share one [128,1024]
          pav tile (head 1 bank-aligned at col 512; accumulation groups must
          not straddle 2KB PSUM banks) so ONE strided copy per chunk (DVE)
          drains both; DMA out every 2 chunks, per-chunk at the drain.
          Input DMAs spread across SP/gpsimd/ACT DGE queues so generation
          latencies overlap (fill) and slab arrivals outpace consumers.

Host (f32 numpy): everything O(N*C): LN folds, convs, l2 norms, top-k
selection (f32-exact like the baseline), fp8/bf16 packing, and the finish:
attn = u/den, z = W_out @ attn, cross-core head-sum, channel-LN, residual.
"""

import numpy as np
import ml_dtypes

import concourse.bass as bass
import concourse.bacc as bacc
import concourse.tile as tile
import concourse.mybir as mybir
from concourse.bass_utils import run_bass_kernel_spmd
from concourse._compat import with_exitstack

BF16 = mybir.dt.bfloat16
F32 = mybir.dt.float32
F8 = mybir.dt.float8e4
U16 = mybir.dt.uint16
bf16 = ml_dtypes.bfloat16
f8e4 = ml_dtypes.float8_e4m3

HEADS, DH, C = 8, 64, 128
D, H, W = 16, 32, 32
N = D * H * W            # 16384 voxels per batch
B = 2
NCORES = 8
KD = KH = KW = 8
NKV = KD * KH * KW       # 512 selected kv positions per head
VCH = 512                # vox chunk
NVC = N // VCH           # 32 chunks
KVC = 128                # kv chunk

LOG2E = float(np.log2(np.e))
SCH_C1 = 128.0 * LOG2E
SCH_C2 = float(127 * 128) - 128.0 * 0.043036

DR = mybir.MatmulPerfMode.DoubleRow
EXP = mybir.ActivationFunctionType.Exp


def _mk_engine_seq(quotas, total):
    """Bresenham-spread engine assignment sequence honoring quotas."""
    seq = []
    acc = {e: 0.0 for e in quotas}
    for _ in range(total):
        for e in quotas:
            acc[e] += quotas[e] / total
        pick = max(acc, key=lambda e: acc[e])
        acc[pick] -= 1.0
        seq.append(pick)
    return seq


# 256 exp tiles ([128, 512] each). HW constraint: GPSIMD/Pool cannot touch
# PSUM, so only ACT (612ns) and DVE (658ns) can consume sim psum tiles; the
# 64 av stage copies (441ns, psum reads) also go to DVE/ACT.
EXP_QUOTAS = {'act': 80, 'dve': 48}
LAG = 2            # chunk-heads of av delay behind sim/exp
PSIM_BUFS = 3
PAV_BUFS = 2
SBEX_BUFS = 8
SIM_PRIO = 0       # >0: emit sim matmuls with high_priority(offset)
GRAIN = 1024       # exp/psum tile grain: 512 or 1024
WARMUP = 0
AVPAIR = 1         # pair both heads of a chunk in one [128,1024] pav tile
                   # (head 1 at col 512: bank-aligned; a [128,520] packing is
                   # illegal -- av groups must not straddle 2KB PSUM banks)
TAILW = 0          # drain-bias window of the engine assignment (0 = off)
SEQ_ROT = 0        # rotate the engine-assignment sequence (schedule jitter)
DRAINW = 0         # drain split off: the shared ring already smooths the drain
STARTW = 1         # chunk-heads at the start in split halves
AV_FIRST = 0       # emit av(i-LAG) before sim_exp(i)
QMODE = 0          # slab queue pattern; UOUT_Q: 0=sync 1=gpsimd
UOUT_Q = 0
SLAB0 = 1          # chunks covered by the first qh DMA
SWAPS = ()         # (idx, engine) overrides of the exp assignment sequence
LASTSPLIT = 0      # last chunk per-head drain: tested worse than paired copy
RINGPOOL = 1       # single shared 4-slot psum ring for sim+av tiles
AVPOS = 0          # av emission phase: 0=after each sim_exp, 1=before even, 2=batched after odd

_EXP_SEQ = None


def _exp_engine(idx):
    global _EXP_SEQ
    if _EXP_SEQ is None:
        total = sum(EXP_QUOTAS.values())
        _EXP_SEQ = _mk_engine_seq(EXP_QUOTAS, total)
        for ix, e in SWAPS:
            _EXP_SEQ[ix] = e
        if SEQ_ROT:
            _EXP_SEQ = _EXP_SEQ[SEQ_ROT:] + _EXP_SEQ[:SEQ_ROT]
        if TAILW:
            # drain bias: within the last TAILW tiles, run DVE's share first
            # and ACT's last -- ACT drains its queue earlier, so the final
            # exps (which gate the last avs) land on the idle engine
            tail = _EXP_SEQ[-TAILW:]
            _EXP_SEQ[-TAILW:] = (
                [e for e in tail if e != 'act'] + [e for e in tail if e == 'act'])
    return _EXP_SEQ[idx % len(_EXP_SEQ)]


# ----------------------------------------------------------------------------
# device program
# ----------------------------------------------------------------------------

@with_exitstack
def _device_kernel(ctx, tc, io):
    nc = tc.nc
    qh_d = io['qh']        # [64, NVC*1024] f8: qhat packed (j, head r, x)
    kf_d = io['kf']        # [64, 2048] f8: per head [kc][r][128kv], zero off-head
    vf_d = io['vf']        # [128, 520] bf16: per head 4 kc-blocks [128kv, 65]
    uout = io['uout']      # [128, NVC*520] bf16 out: u|den per (j, h, vb)

    cpool = ctx.enter_context(tc.tile_pool(name="consts", bufs=1))
    # issue the critical first loads on three different DGE queues so their
    # generation latencies overlap; head A's kf half lands first, vf (only
    # needed by the first av, much later) goes last on the ACT queue
    kf = cpool.tile([64, 2048], F8)
    nc.gpsimd.dma_start(kf[:, 0:1024], kf_d[:, 0:1024])
    qh = cpool.tile([64, NVC * 1024], F8)
    nc.sync.dma_start(qh[:, 0:SLAB0 * 1024], qh_d[:, 0:SLAB0 * 1024])
    nc.gpsimd.dma_start(kf[:, 1024:2048], kf_d[:, 1024:2048])
    vf = cpool.tile([128, 520], BF16)
    nc.scalar.dma_start(vf[:], vf_d[:])
    # remaining qh slabs (first ones small) so the pipeline starts early;
    # alternate SP/gpsimd DGE queues so slab arrivals outpace the consumers
    edges = [SLAB0, 2, 4, 8, 12, 16, 24, 32]
    edges = [e for e in edges if e > SLAB0 or e == SLAB0]
    edges = sorted(set([SLAB0] + [e for e in [2, 4, 8, 12, 16, 24, 32] if e > SLAB0]))
    for s in range(len(edges) - 1):
        lo, hi = edges[s] * 1024, edges[s + 1] * 1024
        if QMODE == 4:
            eng = nc.gpsimd
        elif QMODE == 5:
            eng = nc.gpsimd if s % 2 == 0 else nc.sync
        else:
            eng = nc.sync if s % 2 == 0 else nc.gpsimd
        eng.dma_start(qh[:, lo:hi], qh_d[:, lo:hi])

    # PE pstate warmup: the tensor engine ramps 0.65 -> 2.4 GHz over ~3us of
    # continuous execution. Dummy matmuls on a zeroed scratch tile fill the
    # initial DMA-wait window so the first real sims run at full clock.
    warm = cpool.tile([64, 512], F8)
    nc.gpsimd.memset(warm[:], 0)

    # Software pipeline: av(i) is emitted LAG chunk-heads behind sim/exp(i)
    # so PE's FIFO queue never head-blocks on an exp still in flight.
    if RINGPOOL:
        ring = ctx.enter_context(
            tc.tile_pool(name="ring", bufs=4, space="PSUM"))
        psim = pav = ring
    else:
        psim = ctx.enter_context(
            tc.tile_pool(name="psim", bufs=PSIM_BUFS, space="PSUM"))
        pav = ctx.enter_context(
            tc.tile_pool(name="pav", bufs=(1 if AVPAIR else PAV_BUFS),
                         space="PSUM"))
    with tc.tile_pool(name="sbex", bufs=SBEX_BUFS) as sbex, \
         tc.tile_pool(name="sbst", bufs=3) as sbst:
        exs = {}
        stage = [None]

        avt = [None]

        def emit_exp(eng, exsl, smsl):
            if eng == 'act':
                nc.scalar.activation(exsl, smsl, EXP)
            elif eng == 'dve':
                nc.vector.tensor_scalar(
                    exsl.bitcast(U16), smsl, SCH_C1, SCH_C2,
                    op0=mybir.AluOpType.mult, op1=mybir.AluOpType.add)
            else:
                nc.gpsimd.tensor_scalar(
                    exsl.bitcast(U16), smsl, SCH_C1, SCH_C2,
                    op0=mybir.AluOpType.mult, op1=mybir.AluOpType.add)

        def emit_sim_exp(i):
            j, h = divmod(i, 2)
            rhs = qh[:, j * 1024:(j + 1) * 1024].rearrange(
                "p (two x) -> p two x", two=2)
            kfh = kf[:, h * 1024:(h + 1) * 1024]
            ex = sbex.tile([128, 2048], BF16, tag="ex")
            exs[i] = ex
            if GRAIN == 512:
                # one [128, 512] psum tile (= 1 bank) per kv-chunk with its
                # own exp instr: deepest sim ring (6 slots), each slot frees
                # on a single exp
                for kc in range(4):
                    sm = psim.tile([128, 512], F32, tag="sim")
                    nc.tensor.matmul(
                        sm[:],
                        lhsT=kfh[:, kc * 256:(kc + 1) * 256].rearrange(
                            "p (two m) -> p two m", two=2),
                        rhs=rhs, perf_mode=DR)
                    emit_exp(_exp_engine(4 * i + kc),
                             ex[:, kc * 512:(kc + 1) * 512], sm[:])
            else:
                # [128, 1024] psum tiles (2 banks, kc pairs): one exp instr
                # per tile amortizes the psum/sbuf access init
                drain = i >= NVC * 2 - DRAINW or i < STARTW
                for t in range(2):
                    sm = psim.tile([128, 1024], F32, tag="sim")
                    for kk in range(2):
                        kc = 2 * t + kk
                        nc.tensor.matmul(
                            sm[:, kk * 512:(kk + 1) * 512],
                            lhsT=kfh[:, kc * 256:(kc + 1) * 256].rearrange(
                                "p (two m) -> p two m", two=2),
                            rhs=rhs, perf_mode=DR)
                    if drain:
                        # last chunk-heads: finish in [128, 512] halves spread
                        # over BOTH engines so the final av gate clears early
                        for kk in range(2):
                            emit_exp('act' if (2 * t + kk) % 2 == 0 else 'dve',
                                     ex[:, (2 * t + kk) * 512:
                                        (2 * t + kk + 1) * 512],
                                     sm[:, kk * 512:(kk + 1) * 512])
                    else:
                        emit_exp(_exp_engine(2 * i + t),
                                 ex[:, t * 1024:(t + 1) * 1024], sm[:])

        def emit_av(i):
            j, h = divmod(i, 2)
            if j % 2 == 0 and h == 0:
                stage[0] = sbst.tile([128, 1040], BF16, tag="stage", name="stage")
            ex = exs.pop(i)
            vfh = vf[:, h * 260:(h + 1) * 260]
            if AVPAIR:
                # both heads of chunk j share one [128, 1024] pav tile with
                # head h at column h*512: each 65-col accumulation group stays
                # inside one 2KB PSUM bank (groups must not straddle banks),
                # and ONE strided copy per chunk replaces two copies
                if h == 0:
                    avt[0] = pav.tile([128, 1024], F32,
                                      tag=("sim" if RINGPOOL else "av"),
                                      name="avt")
                av = avt[0][:, h * 512:h * 512 + 260]
            else:
                av = pav.tile([128, 260], F32, tag="av")
            for vb in range(4):
                for kc in range(4):
                    nc.tensor.matmul(
                        av[:, vb * 65:(vb + 1) * 65],
                        lhsT=ex[:, kc * 512 + vb * 128:kc * 512 + (vb + 1) * 128],
                        rhs=vfh[:, kc * 65:(kc + 1) * 65],
                        start=(kc == 0), stop=(kc == 3))
            off = (j % 2) * 520 + h * 260
            if AVPAIR:
                if LASTSPLIT and j == NVC - 1:
                    # final chunk: drain each head as soon as its avs land
                    off2 = (j % 2) * 520 + h * 260
                    nc.scalar.copy(stage[0][:, off2:off2 + 260],
                                   avt[0][:, h * 512:h * 512 + 260])
                    nc.sync.dma_start(
                        uout[:, j * 520 + h * 260:j * 520 + (h + 1) * 260],
                        stage[0][:, off2:off2 + 260])
                    return
                if h == 1:
                    src3 = avt[0][:, 0:1024].rearrange(
                        "p (two x) -> p two x", two=2)[:, :, 0:260]
                    dst3 = stage[0][:, (j % 2) * 520:(j % 2) * 520 + 520].rearrange(
                        "p (two x) -> p two x", two=2)
                    oeng = nc.gpsimd if UOUT_Q else nc.sync
                    if j >= NVC - 2:
                        # drain: copy on ACT (its queue empties first), DMA
                        # per chunk as soon as the copy lands
                        nc.scalar.copy(dst3, src3)
                        oeng.dma_start(
                            uout[:, j * 520:(j + 1) * 520],
                            stage[0][:, (j % 2) * 520:(j % 2) * 520 + 520])
                    else:
                        nc.vector.tensor_copy(dst3, src3)
                        if j % 2 == 1:
                            oeng.dma_start(
                                uout[:, (j - 1) * 520:(j + 1) * 520], stage[0][:])
                return
            if i >= NVC * 2 - 4:
                # drain: copy on ACT (its exp queue empties first) and DMA
                # each chunk-head slice as soon as its copy lands
                nc.scalar.copy(stage[0][:, off:off + 260], av[:])
                nc.sync.dma_start(
                    uout[:, j * 520 + h * 260:j * 520 + (h + 1) * 260],
                    stage[0][:, off:off + 260])
            else:
                nc.vector.tensor_copy(stage[0][:, off:off + 260], av[:])
                if j % 2 == 1 and h == 1:
                    nc.sync.dma_start(
                        uout[:, (j - 1) * 520:(j + 1) * 520], stage[0][:])

        NW = NVC * 2
        for i in range(NW):
            if (AVPOS == 1 and i % 2 == 0 and i >= LAG) or \
               (AV_FIRST and i >= LAG):
                emit_av(i - LAG)
            emit_sim_exp(i)
            if AVPOS == 2:
                if i % 2 == 1:
                    for k in (i - 1, i):
                        if k >= LAG:
                            emit_av(k - LAG)
            elif AVPOS == 1:
                if i % 2 == 1 and i >= LAG:
                    emit_av(i - LAG)
            elif not AV_FIRST and i >= LAG:
                emit_av(i - LAG)
        for i in range(NW - LAG, NW):
            emit_av(i)


def _build_program():
    nc = bacc.Bacc("TRN2", target_bir_lowering=False, debug=False,
                   num_devices=NCORES)
    io = {}

    def inp(name, shape, dt):
        io[name] = nc.dram_tensor(name, shape, dt, kind="ExternalInput").ap()

    inp('qh', [64, NVC * 1024], F8)
    inp('kf', [64, 2048], F8)
    inp('vf', [128, 520], BF16)
    io['uout'] = nc.dram_tensor('uout', [128, NVC * 520], BF16,
                                kind="ExternalOutput").ap()

    with tile.TileContext(nc) as tc:
        _device_kernel(tc, io)
    nc.compile()
    return nc


_NC = None


def _get_program():
    global _NC
    if _NC is None:
        _NC = _build_program()
    return _NC


# ----------------------------------------------------------------------------
# host side
# ----------------------------------------------------------------------------

def _host_prepare(inputs):
    f32 = np.float32
    qs = np.asarray(inputs['query_source'], f32).reshape(B, C, N)
    ctxf = np.asarray(inputs['context'], f32).reshape(B, C, N)
    w_q = np.asarray(inputs['w_q'], f32)
    w_kv = np.asarray(inputs['w_kv'], f32)
    cg = np.asarray(inputs['ctx_gamma'], f32).reshape(C)
    cb = np.asarray(inputs['ctx_beta'], f32).reshape(C)
    qg = np.asarray(inputs['qs_gamma'], f32).reshape(C)
    qb = np.asarray(inputs['qs_beta'], f32).reshape(C)

    w_k, w_v = w_kv[:HEADS * DH], w_kv[HEADS * DH:]

    # f32 reference-equivalent pipeline (LN -> conv -> l2norm -> topk)
    def chan_ln(x, g, b):
        m = x.mean(1, keepdims=True)
        v = x.var(1, keepdims=True)
        return g[None, :, None] * (x - m) / (np.sqrt(v) + f32(1e-6)) + b[None, :, None]

    ctx_ln = chan_ln(ctxf, cg, cb)
    qs_ln = chan_ln(qs, qg, qb)
    k = np.einsum('bcn,oc->bon', ctx_ln, w_k).reshape(B * HEADS, DH, N)
    q = np.einsum('bcn,oc->bon', qs_ln, w_q).reshape(B * HEADS, DH, N)
    v = np.einsum('bcn,oc->bon', ctx_ln, w_v).reshape(B * HEADS, DH, N)

    def l2n(x):
        nn = np.sqrt((x * x).sum(1, keepdims=True))
        return x / np.maximum(nn, f32(1e-12))

    qh, kh = l2n(q), l2n(k)
    qp = qh.sum(2)                               # [16, 64]
    kab = np.abs(kh).reshape(B * HEADS, DH, D, H, W)
    sd = np.einsum('bc,bcd->bd', qp, kab.sum((3, 4)))
    sh = np.einsum('bc,bch->bh', qp, kab.sum((2, 4)))
    sw = np.einsum('bc,bcw->bw', qp, kab.sum((2, 3)))

    def topk(s, kk):
        return np.argsort(-s, axis=1, kind='stable')[:, :kk]

    id_, ih_, iw_ = topk(sd, KD), topk(sh, KH), topk(sw, KW)
    flat = (id_[:, :, None, None] * (H * W) + ih_[:, None, :, None] * W
            + iw_[:, None, None, :]).reshape(B * HEADS, NKV)

    in_maps = []
    for core in range(NCORES):
        b = core // 4
        hA = (core % 4) * 2
        bhs = (b * HEADS + hA, b * HEADS + hA + 1)

        # qhat packed: [64, NVC*1024], col j*1024 + r*512 + x = qh[bh_r, :, j*512+x]
        qpk = np.empty((64, NVC, 2, VCH), f32)
        for r, bh in enumerate(bhs):
            qpk[:, :, r, :] = qh[bh].reshape(DH, NVC, VCH)
        qpk = qpk.reshape(64, NVC * 1024).astype(f8e4)

        # kf packed [64, 2048]: col h*1024 + kc*256 + r*128 + m; head h's khat
        # sits in k-tile slot r==h, the other slot is zero (shared-rhs trick)
        kfp = np.zeros((64, 2, 4, 2, KVC), f32)
        for r, bh in enumerate(bhs):
            kfp[:, r, :, r, :] = kh[bh][:, flat[bh]].reshape(DH, 4, KVC)
        kfp = kfp.reshape(64, 2048).astype(f8e4)

        # vf: per head 4 blocks [128 kv, 65]: cols h*260 + kc*65 + c
        vfp = np.zeros((128, 520), f32)
        for r, bh in enumerate(bhs):
            vsel = v[bh][:, flat[bh]]            # [64, 512]
            for kc in range(4):
                blk = vsel[:, kc * KVC:(kc + 1) * KVC].T   # [128 kv, 64]
                vfp[:, r * 260 + kc * 65: r * 260 + kc * 65 + 64] = blk
                vfp[:, r * 260 + kc * 65 + 64] = 1.0

        in_maps.append({
            'qh': qpk,
            'kf': kfp,
            'vf': vfp.astype(bf16),
        })
    return in_maps, qs


def _host_finish(results, inputs, qs):
    f32 = np.float32
    w_out = np.asarray(inputs['w_out'], f32)
    og = np.asarray(inputs['out_gamma'], f32).reshape(1, C, 1)
    ob = np.asarray(inputs['out_beta'], f32).reshape(1, C, 1)
    gamma = np.asarray(inputs['gamma'], f32).reshape(-1)[0]
    z = np.zeros((B, C, N), f32)
    for core in range(NCORES):
        b = core // 4
        hA = (core % 4) * 2
        u = results[core]['uout'].astype(f32)        # [128, NVC*520]
        u = u.reshape(128, NVC, 2, 4, 65)            # p, j, h, vb, c
        for h in range(2):
            uh = u[:, :, h, :, :]                    # [128, NVC, 4, 65]
            # vox = j*512 + vb*128 + p
            uh = uh.transpose(1, 2, 0, 3).reshape(N, 65)
            attn = uh[:, :64] / uh[:, 64:65]         # [N, 64]
            z[b] += w_out[:, (hA + h) * DH:(hA + h + 1) * DH] @ attn.T
    m = z.mean(1, keepdims=True)
    vv = z.var(1, keepdims=True)
    out = og * (z - m) / (np.sqrt(vv) + f32(1e-6)) + ob
    out = gamma * out + qs
    return out.reshape(B, C, D, H, W).astype(f32)


def kernel(**inputs):
    in_maps, qs = _host_prepare(inputs)
    nc = _get_program()
    res = run_bass_kernel_spmd(nc, in_maps, list(range(NCORES)))
    return _host_finish(res.results, inputs, qs)


if __name__ == '__main__':
    import reference
    ins = {k: np.asarray(v) for k, v in reference.setup_inputs().items()}
    out = kernel(**ins)
    print("kernel output:", out.shape, out.dtype)
